# revision 1
# baseline (speedup 1.0000x reference)
"""GarNet layer kernel for Trainium2 (8 NeuronCores, data-parallel over batch).

Math (per example b):
    w    = exp(-d_av^2)                      [V=128, S=16]
    hi   = w^T @ fi_v / V                    [S, N=64]
    out  = mean_V(w)[:, None] * hi           [S, N] -> flattened [S*N]

Implementation notes:
  - Batch B=4096 is sharded 512/core across 8 cores (pure data parallel).
  - Per example, one fp32 matmul: lhsT = w [V=128, S=16], rhs = fi
    augmented with a constant column of 1/V^2, so PSUM column N holds
    sum_V(w)/V^2 and the final output is just psum[:, :N] * psum[:, N]
    per partition (exactly the reference quantity).
  - Four examples share one PSUM bank at partition offsets {0,32,64,96}
    via tile_position col-tiling, so the epilogue runs on 128-partition
    tiles and the four matmuls overlap in distinct PE column groups.
"""

import numpy as np
from contextlib import ExitStack

import concourse.bass as bass
import concourse.tile as tile
from concourse import mybir
from concourse.bass_utils import run_bass_kernel_spmd

B, V, S, N = 4096, 128, 16, 64
NCORES = 8
BPC = B // NCORES            # examples per core
ONES_VAL = 1.0 / (V * V)     # exact power of two; folds /V^2 into the matmul


def split_multi_waits(nc):
    """The walrus build in this container rejects >1 embedded sem-wait per
    instruction ("Too many sync wait commands" in setupSyncWait). Hoist every
    multi-wait list onto single-wait EventSemaphore instructions immediately
    before the owner on the same engine — identical semantics, since engine
    streams are in order."""
    fn = nc.m.functions[0]
    for block in fn.blocks:
        insts = list(block.instructions)
        changed = False
        new = []
        for inst in insts:
            si = inst.sync_info
            waits = list(si.on_wait) if (si and si.on_wait) else []
            if len(waits) > 1:
                changed = True
                for w in waits:
                    ev = mybir.InstEventSemaphore(
                        name=nc.get_next_instruction_name(), ins=[], outs=[]
                    )
                    ev.engine = inst.engine
                    ev.sync_info = mybir.SyncInfo(on_wait=[w], on_update=[])
                    new.append(ev)
                ups = list(si.on_update) if si.on_update else []
                inst.sync_info = mybir.SyncInfo(on_wait=[], on_update=ups)
            new.append(inst)
        if changed:
            block.instructions = new


def build(bpc=BPC, e_chunk=32, name="garnet", split_waits=True):
    """Build the per-core Bass module for a shard of `bpc` examples.

    split_waits: apply the walrus multi-wait workaround (needed for HW
    compile; leave False for CoreSim, whose race detector doesn't know
    about post-hoc instructions).
    """
    assert bpc % e_chunk == 0 and e_chunk % 8 == 0
    nchunk = bpc // e_chunk
    G = e_chunk // 8   # psum groups (8 examples each) per chunk
    Q = e_chunk // 2   # w pairs per chunk

    nc = bass.Bass(name=name)
    fi = nc.dram_tensor("fi_v", (bpc, V, N), mybir.dt.float32, kind="ExternalInput")
    dav = nc.dram_tensor("d_av", (bpc, V, S), mybir.dt.float32, kind="ExternalInput")
    out = nc.dram_tensor("out", (bpc, S * N), mybir.dt.float32, kind="ExternalOutput")

    f32 = mybir.dt.float32
    with tile.TileContext(nc) as tc, ExitStack() as ctx:
        fipool = ctx.enter_context(tc.tile_pool(name="fipool", bufs=2))
        dpool = ctx.enter_context(tc.tile_pool(name="dpool", bufs=2))
        opool = ctx.enter_context(tc.tile_pool(name="opool", bufs=2))
        colpool = ctx.enter_context(tc.tile_pool(name="colpool", bufs=4))
        psum = ctx.enter_context(tc.tile_pool(name="psum", bufs=8, space="PSUM"))

        for c in range(nchunk):
            b0 = c * e_chunk
            # fi chunk -> [V, e, N+1]; col N = 1/V^2 for the wbar column
            fi_t = fipool.tile([128, e_chunk, N + 1], f32)
            nc.vector.memset(fi_t[:, :, N : N + 1], ONES_VAL)
            nc.sync.dma_start(
                out=fi_t[:, :, 0:N],
                in_=fi[b0 : b0 + e_chunk].rearrange("e v n -> v e n"),
            )
            # d chunk -> [V, pair, slot, S] with slot layout [w_2q, ZERO, w_2q+1];
            # then w = exp(-d^2) on the two w slots only (zeros stay zero).
            # Each matmul then takes a 32-wide lhsT: pair-even = (w_a, Z),
            # pair-odd = (Z, w_b). With PSUM accumulate (start only on the
            # bank's first matmul), the zero half writes/accumulates zeros, so
            # 8 examples pack one bank at rows 16*jj with no junk rows.
            d_t = dpool.tile([128, Q, 3, S], f32)
            nc.vector.memset(d_t[:, :, 1, :], 0.0)
            dsrc = dav[b0 : b0 + e_chunk].rearrange("(q t) v s -> t v q s", t=2)
            for t in range(2):
                nc.sync.dma_start(out=d_t[:, :, 2 * t, :], in_=dsrc[t])
                nc.vector.tensor_mul(
                    d_t[:, :, 2 * t, :], d_t[:, :, 2 * t, :], d_t[:, :, 2 * t, :]
                )
                nc.scalar.activation(
                    d_t[:, :, 2 * t, :],
                    d_t[:, :, 2 * t, :],
                    mybir.ActivationFunctionType.Exp,
                    scale=-1.0,
                )

            o_t = opool.tile([128, G, N], f32)
            for g in range(G):
                ps = psum.tile([128, N + 1], f32)
                for jj in range(8):
                    e = g * 8 + jj          # example within chunk
                    q, t = e // 2, e % 2    # pair index, parity
                    nc.tensor.matmul(
                        out=ps[32 * (jj // 2) : 32 * (jj // 2) + 32, :],
                        lhsT=d_t[:, q, t : t + 2, :],
                        rhs=fi_t[:, e, :],
                        start=(t == 0),
                        stop=(t == 1),
                        tile_position=(0, 32 * (jj // 2)),
                    )
                col = colpool.tile([128, 1], f32)
                nc.scalar.copy(col, ps[:, N : N + 1])
                nc.vector.tensor_scalar_mul(o_t[:, g, :], ps[:, 0:N], col)

            # partition p = 16*jj + s maps linearly to DRAM offset p*256B of
            # example b0+8g+jj -> one full-128-partition DMA per chunk.
            dst = out[b0 : b0 + e_chunk].rearrange(
                "(g jj) (s n) -> (jj s) g n", jj=8, s=S
            )
            nc.sync.dma_start(out=dst, in_=o_t)

    if split_waits:
        split_multi_waits(nc)
    return nc


_NC_CACHE = {}


def _get_nc():
    if "nc" not in _NC_CACHE:
        _NC_CACHE["nc"] = build()
    return _NC_CACHE["nc"]


def kernel(fi_v: np.ndarray, d_av: np.ndarray) -> np.ndarray:
    fi_v = np.ascontiguousarray(np.asarray(fi_v, dtype=np.float32))
    d_av = np.ascontiguousarray(np.asarray(d_av, dtype=np.float32))
    assert fi_v.shape == (B, V, N) and d_av.shape == (B, V, S)
    nc = _get_nc()
    in_maps = [
        {
            "fi_v": fi_v[c * BPC : (c + 1) * BPC],
            "d_av": d_av[c * BPC : (c + 1) * BPC],
        }
        for c in range(NCORES)
    ]
    res = run_bass_kernel_spmd(nc, in_maps, core_ids=list(range(NCORES)))
    return np.concatenate([res.results[c]["out"] for c in range(NCORES)], axis=0)



# revision 5
# speedup vs baseline: 2.1961x; 2.1961x over previous
"""GarNet layer kernel for Trainium2 (8 NeuronCores, data-parallel over batch).

Math (per example b):
    w    = exp(-d_av^2)                      [V=128, S=16]
    hi   = w^T @ fi_v / V                    [S, N=64]
    out  = mean_V(w)[:, None] * hi           [S, N] -> flattened [S*N]

Implementation notes (v2 — engine-balanced, pair-packed):
  - Batch B=4096 is sharded 512/core across 8 cores (pure data parallel).
  - fi is loaded in a v-pair layout: partition P<64 holds even example
    e0=2u, P>=64 holds odd example e1=2u+1, with v = 2*(P%64)+t for slot
    t in {0,1}. Each (e, v-pair) is a 512-byte contiguous DRAM run, so
    the fi DMA runs at full modeled bandwidth (no <512B penalty).
  - One fp32 matmul per (pair, slot): the stationary operand is a
    block-diagonal [128, 32] tile (w(e0) in cols 0:16 on P<64, w(e1) in
    cols 16:32 on P>=64, zeros elsewhere), so a single 64-row moving
    stream computes BOTH examples of the pair. A second 1-row matmul
    against a constant 1/V^2 column accumulates sum_V(w)/V^2 into psum
    col 64, and the epilogue is psum[:, :64] * psum[:, 64] per partition.
  - DMAs are spread across engines (fi on SP, d_av + out on GpSimd/Pool)
    so no single engine serializes all the HBM traffic.
"""

import numpy as np
from contextlib import ExitStack

import concourse.bass as bass
import concourse.tile as tile
from concourse import mybir
from concourse.bass_utils import run_bass_kernel_spmd

B, V, S, N = 4096, 128, 16, 64
NCORES = 8
BPC = B // NCORES            # examples per core
ONES_VAL = 1.0 / (V * V)     # exact power of two; folds /V^2 into the matmul


def split_multi_waits(nc):
    """The walrus build in this container rejects >1 embedded sem-wait per
    instruction ("Too many sync wait commands" in setupSyncWait). Hoist every
    multi-wait list onto single-wait EventSemaphore instructions immediately
    before the owner on the same engine — identical semantics, since engine
    streams are in order."""
    fn = nc.m.functions[0]
    for block in fn.blocks:
        insts = list(block.instructions)
        changed = False
        new = []
        for inst in insts:
            si = inst.sync_info
            waits = list(si.on_wait) if (si and si.on_wait) else []
            if len(waits) > 1:
                changed = True
                for w in waits:
                    ev = mybir.InstEventSemaphore(
                        name=nc.get_next_instruction_name(), ins=[], outs=[]
                    )
                    ev.engine = inst.engine
                    ev.sync_info = mybir.SyncInfo(on_wait=[w], on_update=[])
                    new.append(ev)
                ups = list(si.on_update) if si.on_update else []
                inst.sync_info = mybir.SyncInfo(on_wait=[], on_update=ups)
            new.append(inst)
        if changed:
            block.instructions = new


def build(bpc=BPC, e_chunk=32, name="garnet", split_waits=True):
    """Build the per-core Bass module for a shard of `bpc` examples.

    split_waits: apply the walrus multi-wait workaround (needed for HW
    compile; leave False for CoreSim, whose race detector doesn't know
    about post-hoc instructions).
    """
    assert bpc % e_chunk == 0 and e_chunk % 8 == 0
    nchunk = bpc // e_chunk
    U = e_chunk // 2   # example pairs per chunk
    G = e_chunk // 8   # psum banks (8 examples = 4 pairs each) per chunk

    nc = bass.Bass(name=name)
    fi = nc.dram_tensor("fi_v", (bpc, V, N), mybir.dt.float32, kind="ExternalInput")
    dav = nc.dram_tensor("d_av", (bpc, V, S), mybir.dt.float32, kind="ExternalInput")
    out = nc.dram_tensor("out", (bpc, S * N), mybir.dt.float32, kind="ExternalOutput")

    f32 = mybir.dt.float32
    with tile.TileContext(nc) as tc, ExitStack() as ctx:
        fipool = ctx.enter_context(tc.tile_pool(name="fipool", bufs=2))
        dpool = ctx.enter_context(tc.tile_pool(name="dpool", bufs=2))
        opool = ctx.enter_context(tc.tile_pool(name="opool", bufs=2))
        colpool = ctx.enter_context(tc.tile_pool(name="colpool", bufs=4))
        constpool = ctx.enter_context(tc.tile_pool(name="constpool", bufs=1))
        wpool = ctx.enter_context(tc.tile_pool(name="wpool", bufs=1))
        psum = ctx.enter_context(tc.tile_pool(name="psum", bufs=8, space="PSUM"))

        # Constant 1/V^2 column for the wbar matmul.
        ones_t = constpool.tile([128, 1], f32)
        nc.vector.memset(ones_t, ONES_VAL)

        # Persistent double-buffered stationary-w ring. Block-diagonal
        # padding: the off-diagonal blocks are zeroed ONCE here and never
        # rewritten; exp() only writes the diagonal blocks each chunk.
        w_ring = wpool.tile([128, 2, U, 2, 32], f32)
        nc.vector.memset(w_ring[0:64, :, :, :, 16:32], 0.0)
        nc.vector.memset(w_ring[64:128, :, :, :, 0:16], 0.0)

        for c in range(nchunk):
            b0 = c * e_chunk
            wbuf = c % 2

            # fi chunk -> [128, U, 128]: partition P = 64*e2 + p holds
            # fi[b0+2u+e2, 2p+t, n] at free offset (u, 64t+n). Every
            # (partition, u) is one 512B contiguous DRAM run -> full-rate DMA.
            fi_t = fipool.tile([128, U, 128], f32)
            nc.sync.dma_start(
                out=fi_t,
                in_=fi[b0 : b0 + e_chunk].rearrange(
                    "(u e2) (p vp) n -> (e2 p) u (vp n)", e2=2, vp=2
                ),
            )

            # d chunk -> [128, U, 2, 16] in the same (e-parity, v-pair) layout.
            d_t = dpool.tile([128, U, 2, 16], f32)
            nc.gpsimd.dma_start(
                out=d_t,
                in_=dav[b0 : b0 + e_chunk].rearrange(
                    "(u e2) (p vp) s -> (e2 p) u (vp s)", e2=2, vp=2
                ),
            )

            # w = exp(-d^2) into the block-diagonal stationary tiles.
            nc.vector.tensor_mul(d_t, d_t, d_t)
            nc.scalar.activation(
                w_ring[0:64, wbuf, :, :, 0:16],
                d_t[0:64],
                mybir.ActivationFunctionType.Exp,
                scale=-1.0,
            )
            nc.scalar.activation(
                w_ring[64:128, wbuf, :, :, 16:32],
                d_t[64:128],
                mybir.ActivationFunctionType.Exp,
                scale=-1.0,
            )

            o_t = opool.tile([128, G, N], f32)
            for g in range(G):
                ps = psum.tile([128, N + 1], f32)
                for j in range(4):
                    u = g * 4 + j
                    # One accumulation group per 32-row block covering BOTH
                    # the [0:N] output cols and the wbar col N: start only on
                    # the first matmul, stop only on the last, so the 2KB
                    # psum zero region is started exactly once.
                    for t in range(2):
                        nc.tensor.matmul(
                            out=ps[32 * j : 32 * j + 32, 0:N],
                            lhsT=w_ring[:, wbuf, u, t, :],
                            rhs=fi_t[:, u, 64 * t : 64 * t + 64],
                            start=(t == 0),
                            stop=False,
                            tile_position=(0, 32 * j),
                            skip_group_check=True,
                        )
                        nc.tensor.matmul(
                            out=ps[32 * j : 32 * j + 32, N : N + 1],
                            lhsT=w_ring[:, wbuf, u, t, :],
                            rhs=ones_t,
                            start=False,
                            stop=(t == 1),
                            tile_position=(0, 32 * j),
                            skip_group_check=True,
                        )
                col = colpool.tile([128, 1], f32)
                nc.scalar.copy(col, ps[:, N : N + 1])
                nc.vector.tensor_scalar_mul(o_t[:, g, :], ps[:, 0:N], col)

            # psum partition 32j+16t+s belongs to example b0+8g+2j+t, row s.
            nc.gpsimd.dma_start(
                out=out[b0 : b0 + e_chunk].rearrange(
                    "(g j t) (s n) -> (j t s) g n", j=4, t=2, s=S
                ),
                in_=o_t,
            )

    if split_waits:
        split_multi_waits(nc)
    return nc


_NC_CACHE = {}


def _get_nc():
    if "nc" not in _NC_CACHE:
        _NC_CACHE["nc"] = build()
    return _NC_CACHE["nc"]


def kernel(fi_v: np.ndarray, d_av: np.ndarray) -> np.ndarray:
    fi_v = np.ascontiguousarray(np.asarray(fi_v, dtype=np.float32))
    d_av = np.ascontiguousarray(np.asarray(d_av, dtype=np.float32))
    assert fi_v.shape == (B, V, N) and d_av.shape == (B, V, S)
    nc = _get_nc()
    in_maps = [
        {
            "fi_v": fi_v[c * BPC : (c + 1) * BPC],
            "d_av": d_av[c * BPC : (c + 1) * BPC],
        }
        for c in range(NCORES)
    ]
    res = run_bass_kernel_spmd(nc, in_maps, core_ids=list(range(NCORES)))
    return np.concatenate([res.results[c]["out"] for c in range(NCORES)], axis=0)


# revision 17
# speedup vs baseline: 2.2601x; 1.0292x over previous
"""GarNet layer kernel for Trainium2 (8 NeuronCores, data-parallel over batch).

Math (per example b):
    w    = exp(-d_av^2)                      [V=128, S=16]
    hi   = w^T @ fi_v / V                    [S, N=64]
    out  = mean_V(w)[:, None] * hi           [S, N] -> flattened [S*N]

Implementation notes (v2 — engine-balanced, pair-packed):
  - Batch B=4096 is sharded 512/core across 8 cores (pure data parallel).
  - fi is loaded in a v-pair layout: partition P<64 holds even example
    e0=2u, P>=64 holds odd example e1=2u+1, with v = 2*(P%64)+t for slot
    t in {0,1}. Each (e, v-pair) is a 512-byte contiguous DRAM run, so
    the fi DMA runs at full modeled bandwidth (no <512B penalty).
  - One fp32 matmul per (pair, slot): the stationary operand is a
    block-diagonal [128, 32] tile (w(e0) in cols 0:16 on P<64, w(e1) in
    cols 16:32 on P>=64, zeros elsewhere), so a single 64-row moving
    stream computes BOTH examples of the pair. A second 1-row matmul
    against a constant 1/V^2 column accumulates sum_V(w)/V^2 into psum
    col 64, and the epilogue is psum[:, :64] * psum[:, 64] per partition.
  - DMAs are spread across engines (fi on SP, d_av + out on GpSimd/Pool)
    so no single engine serializes all the HBM traffic.
"""

import numpy as np
from contextlib import ExitStack

import concourse.bass as bass
import concourse.tile as tile
from concourse import mybir
from concourse.bass_utils import run_bass_kernel_spmd

B, V, S, N = 4096, 128, 16, 64
NCORES = 8
BPC = B // NCORES            # examples per core
ONES_VAL = 1.0 / (V * V)     # exact power of two; folds /V^2 into the matmul


def split_multi_waits(nc):
    """The walrus build in this container rejects >1 embedded sem-wait per
    instruction ("Too many sync wait commands" in setupSyncWait). Hoist every
    multi-wait list onto single-wait EventSemaphore instructions immediately
    before the owner on the same engine — identical semantics, since engine
    streams are in order."""
    fn = nc.m.functions[0]
    for block in fn.blocks:
        insts = list(block.instructions)
        changed = False
        new = []
        for inst in insts:
            si = inst.sync_info
            waits = list(si.on_wait) if (si and si.on_wait) else []
            if len(waits) > 1:
                changed = True
                for w in waits:
                    ev = mybir.InstEventSemaphore(
                        name=nc.get_next_instruction_name(), ins=[], outs=[]
                    )
                    ev.engine = inst.engine
                    ev.sync_info = mybir.SyncInfo(on_wait=[w], on_update=[])
                    new.append(ev)
                ups = list(si.on_update) if si.on_update else []
                inst.sync_info = mybir.SyncInfo(on_wait=[], on_update=ups)
            new.append(inst)
        if changed:
            block.instructions = new


def build(bpc=BPC, e_chunk=32, name="garnet", split_waits=True, fi_bufs=3, wb=3,
          warm=True, edge=8, warm_rows=256, warm_n=4):
    """Build the per-core Bass module for a shard of `bpc` examples."""
    # Chunk plan: small first/last chunks shrink pipeline fill and drain.
    plan = []
    rem = bpc
    if edge and bpc > 2 * e_chunk:
        first = [edge, e_chunk - edge]
        last = [e_chunk - edge, edge]
        mid = (bpc - 2 * e_chunk) // e_chunk
        plan = first + [e_chunk] * mid + last
    else:
        plan = [e_chunk] * (bpc // e_chunk)
    assert sum(plan) == bpc and all(p % 8 == 0 for p in plan)
    U = e_chunk // 2          # max pairs per chunk (pool slot size)

    nc = bass.Bass(name=name)
    fi = nc.dram_tensor("fi_v", (bpc, V, N), mybir.dt.float32, kind="ExternalInput")
    dav = nc.dram_tensor("d_av", (bpc, V, S), mybir.dt.float32, kind="ExternalInput")
    out = nc.dram_tensor("out", (bpc, S * N), mybir.dt.float32, kind="ExternalOutput")

    f32 = mybir.dt.float32
    with tile.TileContext(nc) as tc, ExitStack() as ctx:
        fipool = ctx.enter_context(tc.tile_pool(name="fipool", bufs=fi_bufs))
        dpool = ctx.enter_context(tc.tile_pool(name="dpool", bufs=2))
        opool = ctx.enter_context(tc.tile_pool(name="opool", bufs=2))
        colpool = ctx.enter_context(tc.tile_pool(name="colpool", bufs=4))
        constpool = ctx.enter_context(tc.tile_pool(name="constpool", bufs=1))
        wpool = ctx.enter_context(tc.tile_pool(name="wpool", bufs=1))
        psum = ctx.enter_context(
            tc.tile_pool(name="psum", bufs=(7 if warm_rows and warm_n else 8), space="PSUM")
        )
        psumw = (
            ctx.enter_context(tc.tile_pool(name="psumw", bufs=1, space="PSUM"))
            if warm_rows and warm_n
            else None
        )

        WB = wb
        ones_t = constpool.tile([128, 1], f32)
        nc.vector.memset(ones_t, ONES_VAL)

        if warm:
            # Load the Act engine's Exp table during chunk-0 DMAs instead of
            # on the first real exp.
            warm_t = constpool.tile([128, 1], f32)
            nc.scalar.activation(
                warm_t, ones_t, mybir.ActivationFunctionType.Exp, scale=-1.0
            )

        # Persistent stationary-w ring. Off-diagonal zero blocks are memset
        # once; buffer 0 first so chunk 0 is not blocked behind the full-ring
        # memset (the tile scheduler keeps per-engine program order).
        w_ring = wpool.tile([128, WB, U, 2, 32], f32)
        nc.vector.memset(w_ring[0:64, 0, :, :, 16:32], 0.0)
        nc.vector.memset(w_ring[64:128, 0, :, :, 0:16], 0.0)

        starts = [sum(plan[:i]) for i in range(len(plan))]

        def load_fi(c):
            uc = plan[c] // 2
            b0 = starts[c]
            fi_t = fipool.tile([128, U, 128], f32)
            nc.sync.dma_start(
                out=fi_t[:, 0:uc, :],
                in_=fi[b0 : b0 + plan[c]].rearrange(
                    "(u e2) (p vp) n -> (e2 p) u (vp n)", e2=2, vp=2
                ),
            )
            return fi_t

        def load_w(c):
            uc = plan[c] // 2
            b0 = starts[c]
            wbuf = c % WB
            d_t = dpool.tile([128, U, 2, 16], f32)
            (nc.sync if c == 0 else nc.gpsimd).dma_start(
                out=d_t[:, 0:uc, :, :],
                in_=dav[b0 : b0 + plan[c]].rearrange(
                    "(u e2) (p vp) s -> (e2 p) u (vp s)", e2=2, vp=2
                ),
            )
            nc.vector.tensor_mul(d_t[:, 0:uc], d_t[:, 0:uc], d_t[:, 0:uc])
            nc.scalar.activation(
                w_ring[0:64, wbuf, 0:uc, :, 0:16],
                d_t[0:64, 0:uc],
                mybir.ActivationFunctionType.Exp,
                scale=-1.0,
            )
            nc.scalar.activation(
                w_ring[64:128, wbuf, 0:uc, :, 16:32],
                d_t[64:128, 0:uc],
                mybir.ActivationFunctionType.Exp,
                scale=-1.0,
            )
            return d_t

        # PE p-state warmup: dummy matmuls keep the PE busy through the
        # chunk-0 fill so the ramp (full clock after 3us of activity) is
        # complete before the first real matmul. warm_rows*warm_n ~ fill ns.
        if warm_rows and warm_n:
            dummy_t = constpool.tile([128, warm_rows], f32)
            nc.gpsimd.memset(dummy_t, 0.0)
            wps = psumw.tile([128, N + 1], f32)
            for _ in range(warm_n):
                nc.tensor.matmul(
                    out=wps[0:1, 0:warm_rows],
                    lhsT=ones_t,
                    rhs=dummy_t,
                    start=True,
                    stop=True,
                    skip_group_check=True,
                )

        load_w(0)
        fi_tiles = {0: load_fi(0)}
        # Remaining ring buffers' zero blocks: after chunk 0's critical path.
        if WB > 1:
            nc.vector.memset(w_ring[0:64, 1:WB, :, :, 16:32], 0.0)
            nc.vector.memset(w_ring[64:128, 1:WB, :, :, 0:16], 0.0)

        for c in range(len(plan)):
            b0 = starts[c]
            wbuf = c % WB
            G = plan[c] // 8
            if c + 1 < len(plan):
                load_w(c + 1)
                fi_tiles[c + 1] = load_fi(c + 1)
            fi_t = fi_tiles.pop(c)

            o_t = opool.tile([128, G, N], f32)
            for g in range(G):
                ps = psum.tile([128, N + 1], f32)
                for j in range(4):
                    u = g * 4 + j
                    for t in range(2):
                        nc.tensor.matmul(
                            out=ps[32 * j : 32 * j + 32, 0:N],
                            lhsT=w_ring[:, wbuf, u, t, :],
                            rhs=fi_t[:, u, 64 * t : 64 * t + 64],
                            start=(t == 0),
                            stop=False,
                            tile_position=(0, 32 * j),
                            skip_group_check=True,
                        )
                        nc.tensor.matmul(
                            out=ps[32 * j : 32 * j + 32, N : N + 1],
                            lhsT=w_ring[:, wbuf, u, t, :],
                            rhs=ones_t,
                            start=False,
                            stop=(t == 1),
                            tile_position=(0, 32 * j),
                            skip_group_check=True,
                        )
                col = colpool.tile([128, 1], f32)
                nc.scalar.copy(col, ps[:, N : N + 1])
                nc.vector.tensor_scalar_mul(o_t[:, g, :], ps[:, 0:N], col)

            nc.gpsimd.dma_start(
                out=out[b0 : b0 + plan[c]].rearrange(
                    "(g j t) (s n) -> (j t s) g n", j=4, t=2, s=S
                ),
                in_=o_t[:, 0:G, :],
            )

    if split_waits:
        split_multi_waits(nc)
    return nc


_NC_CACHE = {}


def _get_nc():
    if "nc" not in _NC_CACHE:
        _NC_CACHE["nc"] = build()
    return _NC_CACHE["nc"]


def kernel(fi_v: np.ndarray, d_av: np.ndarray) -> np.ndarray:
    fi_v = np.ascontiguousarray(np.asarray(fi_v, dtype=np.float32))
    d_av = np.ascontiguousarray(np.asarray(d_av, dtype=np.float32))
    assert fi_v.shape == (B, V, N) and d_av.shape == (B, V, S)
    nc = _get_nc()
    in_maps = [
        {
            "fi_v": fi_v[c * BPC : (c + 1) * BPC],
            "d_av": d_av[c * BPC : (c + 1) * BPC],
        }
        for c in range(NCORES)
    ]
    res = run_bass_kernel_spmd(nc, in_maps, core_ids=list(range(NCORES)))
    return np.concatenate([res.results[c]["out"] for c in range(NCORES)], axis=0)


# revision 31
# speedup vs baseline: 2.3563x; 1.0426x over previous
"""GarNet layer kernel for Trainium2 (8 NeuronCores, data-parallel over batch).

Math (per example b):
    w    = exp(-d_av^2)                      [V=128, S=16]
    hi   = w^T @ fi_v / V                    [S, N=64]
    out  = mean_V(w)[:, None] * hi           [S, N] -> flattened [S*N]

Implementation notes (engine-balanced, pair-packed):
  - Batch B=4096 is sharded 512/core across 8 cores (pure data parallel).
  - fi is loaded in a v-pair layout: partition P<64 holds even example
    e0=2u, P>=64 holds odd example e1=2u+1, with v = 2*(P%64)+t for slot
    t in {0,1}. Each (e, v-pair) is a 512-byte contiguous DRAM run, so
    the fi DMA runs at full modeled bandwidth (no <512B penalty).
  - One fp32 matmul per (pair, slot): the stationary operand is a
    block-diagonal [128, 32] tile (w(e0) in cols 0:16 on P<64, w(e1) in
    cols 16:32 on P>=64, zeros elsewhere), so a single 64-row moving
    stream computes BOTH examples of the pair. A second 1-row matmul
    against a constant 1/V^2 column accumulates sum_V(w)/V^2 into psum
    col 64, and the epilogue is psum[:, :64] * psum[:, 64] per partition.
  - DMAs are spread across engines (fi on SP, d_av + out on GpSimd/Pool,
    a few startup transfers on Act) so no single engine serializes the
    HBM traffic; the PE (55.5us of fp32 matmul, the structural floor) is
    the only bottleneck engine.
  - Pipeline shaping: small first/last chunks (8/24/24/...32.../8), a
    PE p-state warmup (dummy matmuls so the clock is at 2.4GHz before the
    first real matmul), an early Exp-table preload on Act, and a 3-deep
    stationary-w ring keep the PE gap-free from ~3.4us to the drain.
"""

import numpy as np
from contextlib import ExitStack

import concourse.bass as bass
import concourse.tile as tile
from concourse import mybir
from concourse.bass_utils import run_bass_kernel_spmd

B, V, S, N = 4096, 128, 16, 64
NCORES = 8
BPC = B // NCORES            # examples per core
ONES_VAL = 1.0 / (V * V)     # exact power of two; folds /V^2 into the matmul


def split_multi_waits(nc):
    """The walrus build in this container rejects >1 embedded sem-wait per
    instruction ("Too many sync wait commands" in setupSyncWait). Hoist every
    multi-wait list onto single-wait EventSemaphore instructions immediately
    before the owner on the same engine — identical semantics, since engine
    streams are in order."""
    fn = nc.m.functions[0]
    for block in fn.blocks:
        insts = list(block.instructions)
        changed = False
        new = []
        for inst in insts:
            si = inst.sync_info
            waits = list(si.on_wait) if (si and si.on_wait) else []
            if len(waits) > 1:
                changed = True
                for w in waits:
                    ev = mybir.InstEventSemaphore(
                        name=nc.get_next_instruction_name(), ins=[], outs=[]
                    )
                    ev.engine = inst.engine
                    ev.sync_info = mybir.SyncInfo(on_wait=[w], on_update=[])
                    new.append(ev)
                ups = list(si.on_update) if si.on_update else []
                inst.sync_info = mybir.SyncInfo(on_wait=[], on_update=ups)
            new.append(inst)
        if changed:
            block.instructions = new


def build(bpc=BPC, e_chunk=32, name="garnet", split_waits=True, fi_bufs=3, wb=3,
          warm=True, edge=8, warm_rows=48, warm_n=18,
          fi_act_set=(0,), fi_bufs_override=None, plan=None):
    """Build the per-core Bass module for a shard of `bpc` examples."""
    # Chunk plan: small first/last chunks shrink pipeline fill and drain.
    if plan is None:
        if edge and bpc > 3 * e_chunk:
            # Ramp in with three partial chunks (short fill), ramp out with
            # one small chunk (short drain); tuned on the CoreSim timeline.
            first = [edge, e_chunk - edge, e_chunk - edge]
            mid = (bpc - sum(first) - edge) // e_chunk
            plan = first + [e_chunk] * mid + [edge]
            rem = bpc - sum(plan)
            if rem:
                plan.insert(3, rem)
        else:
            plan = [e_chunk] * (bpc // e_chunk)
    assert sum(plan) == bpc and all(p % 8 == 0 for p in plan)
    assert all(p <= e_chunk for p in plan)
    U = e_chunk // 2          # max pairs per chunk (pool slot size)

    nc = bass.Bass(name=name)
    fi = nc.dram_tensor("fi_v", (bpc, V, N), mybir.dt.float32, kind="ExternalInput")
    dav = nc.dram_tensor("d_av", (bpc, V, S), mybir.dt.float32, kind="ExternalInput")
    out = nc.dram_tensor("out", (bpc, S * N), mybir.dt.float32, kind="ExternalOutput")

    f32 = mybir.dt.float32
    with tile.TileContext(nc) as tc, ExitStack() as ctx:
        fipool = ctx.enter_context(
            tc.tile_pool(name="fipool", bufs=fi_bufs_override or fi_bufs)
        )
        dpool = ctx.enter_context(tc.tile_pool(name="dpool", bufs=2))
        opool = ctx.enter_context(tc.tile_pool(name="opool", bufs=3))
        colpool = ctx.enter_context(tc.tile_pool(name="colpool", bufs=4))
        constpool = ctx.enter_context(tc.tile_pool(name="constpool", bufs=1))
        wpool = ctx.enter_context(tc.tile_pool(name="wpool", bufs=1))
        psum = ctx.enter_context(
            tc.tile_pool(name="psum", bufs=(7 if warm_rows and warm_n else 8), space="PSUM")
        )
        psumw = (
            ctx.enter_context(tc.tile_pool(name="psumw", bufs=1, space="PSUM"))
            if warm_rows and warm_n
            else None
        )

        WB = wb
        ones_t = constpool.tile([128, 1], f32)
        nc.vector.memset(ones_t, ONES_VAL)

        if warm:
            # Load the Act engine's Exp table during chunk-0 DMAs instead of
            # on the first real exp.
            warm_t = constpool.tile([128, 1], f32)
            nc.scalar.activation(
                warm_t, ones_t, mybir.ActivationFunctionType.Exp, scale=-1.0
            )

        # Persistent stationary-w ring. Off-diagonal zero blocks are memset
        # once; buffer 0 first so chunk 0 is not blocked behind the full-ring
        # memset (the tile scheduler keeps per-engine program order).
        w_ring = wpool.tile([128, WB, U, 2, 32], f32)
        nc.vector.memset(w_ring[0:64, 0, :, :, 16:32], 0.0)
        nc.vector.memset(w_ring[64:128, 0, :, :, 0:16], 0.0)

        starts = [sum(plan[:i]) for i in range(len(plan))]
        dma_eng = [nc.sync, nc.gpsimd]

        def load_fi(c):
            uc = plan[c] // 2
            b0 = starts[c]
            fi_t = fipool.tile([128, U, 128], f32)
            (nc.scalar if c in fi_act_set else nc.sync).dma_start(
                out=fi_t[:, 0:uc, :],
                in_=fi[b0 : b0 + plan[c]].rearrange(
                    "(u e2) (p vp) n -> (e2 p) u (vp n)", e2=2, vp=2
                ),
            )
            return fi_t

        def load_w(c):
            uc = plan[c] // 2
            b0 = starts[c]
            wbuf = c % WB
            d_t = dpool.tile([128, U, 2, 16], f32)
            dma_eng[0 if c == 0 else 1].dma_start(
                out=d_t[:, 0:uc, :, :],
                in_=dav[b0 : b0 + plan[c]].rearrange(
                    "(u e2) (p vp) s -> (e2 p) u (vp s)", e2=2, vp=2
                ),
            )
            nc.vector.tensor_mul(d_t[:, 0:uc], d_t[:, 0:uc], d_t[:, 0:uc])
            nc.scalar.activation(
                w_ring[0:64, wbuf, 0:uc, :, 0:16],
                d_t[0:64, 0:uc],
                mybir.ActivationFunctionType.Exp,
                scale=-1.0,
            )
            nc.scalar.activation(
                w_ring[64:128, wbuf, 0:uc, :, 16:32],
                d_t[64:128, 0:uc],
                mybir.ActivationFunctionType.Exp,
                scale=-1.0,
            )
            return d_t

        # PE p-state warmup: dummy matmuls keep the PE busy through the
        # chunk-0 fill so the ramp (full clock after 3us of activity) is
        # complete before the first real matmul. warm_rows*warm_n ~ fill ns.
        if warm_rows and warm_n:
            dummy_t = constpool.tile([128, warm_rows], f32)
            nc.gpsimd.memset(dummy_t, 0.0)
            wps = psumw.tile([128, N + 1], f32)
            for _ in range(warm_n):
                nc.tensor.matmul(
                    out=wps[0:1, 0:warm_rows],
                    lhsT=ones_t,
                    rhs=dummy_t,
                    start=True,
                    stop=True,
                    skip_group_check=True,
                )

        fi_tiles = {0: load_fi(0)}
        load_w(0)

        chunk_otile = {}
        for c in range(len(plan)):
            wbuf = c % WB
            G = plan[c] // 8
            if c + 1 < len(plan):
                load_w(c + 1)
                fi_tiles[c + 1] = load_fi(c + 1)
            if c + 1 < WB:
                # Ring buffer c+1's zero blocks: one buffer per iteration,
                # emitted after that chunk's loads so they never delay the
                # d->sq->exp critical chain (Pool has slack here).
                nc.gpsimd.memset(w_ring[0:64, c + 1, :, :, 16:32], 0.0)
                nc.gpsimd.memset(w_ring[64:128, c + 1, :, :, 0:16], 0.0)
            fi_t = fi_tiles.pop(c)
            o_t = opool.tile([128, G, N], f32)

            for g in range(G):
                ps = psum.tile([128, N + 1], f32)
                for j in range(4):
                    u = g * 4 + j
                    # One accumulation group per 32-row block covering BOTH
                    # the [0:N] cols and the wbar col N: start only on the
                    # first matmul, stop only on the last, so each 2KB psum
                    # zero region is started exactly once.
                    for t in range(2):
                        nc.tensor.matmul(
                            out=ps[32 * j : 32 * j + 32, 0:N],
                            lhsT=w_ring[:, wbuf, u, t, :],
                            rhs=fi_t[:, u, 64 * t : 64 * t + 64],
                            start=(t == 0),
                            stop=False,
                            tile_position=(0, 32 * j),
                            skip_group_check=True,
                        )
                        nc.tensor.matmul(
                            out=ps[32 * j : 32 * j + 32, N : N + 1],
                            lhsT=w_ring[:, wbuf, u, t, :],
                            rhs=ones_t,
                            start=False,
                            stop=(t == 1),
                            tile_position=(0, 32 * j),
                            skip_group_check=True,
                        )
                col = colpool.tile([128, 1], f32)
                nc.scalar.copy(col, ps[:, N : N + 1])
                nc.vector.tensor_scalar_mul(o_t[:, g, :], ps[:, 0:N], col)

            # psum partition 32j+16t+s belongs to example b0+8g+2j+t, row s.
            # The last chunk's store goes via SP (idle by then, and with a
            # slightly shorter DGE init) to shorten the drain.
            (nc.sync if c == len(plan) - 1 else nc.gpsimd).dma_start(
                out=out[starts[c] : starts[c] + plan[c]].rearrange(
                    "(g j t) (s n) -> (j t s) g n", j=4, t=2, s=S
                ),
                in_=o_t,
            )

    if split_waits:
        split_multi_waits(nc)
    return nc


_NC_CACHE = {}


def _get_nc():
    if "nc" not in _NC_CACHE:
        _NC_CACHE["nc"] = build()
    return _NC_CACHE["nc"]


def kernel(fi_v: np.ndarray, d_av: np.ndarray) -> np.ndarray:
    fi_v = np.ascontiguousarray(np.asarray(fi_v, dtype=np.float32))
    d_av = np.ascontiguousarray(np.asarray(d_av, dtype=np.float32))
    assert fi_v.shape == (B, V, N) and d_av.shape == (B, V, S)
    nc = _get_nc()
    in_maps = [
        {
            "fi_v": fi_v[c * BPC : (c + 1) * BPC],
            "d_av": d_av[c * BPC : (c + 1) * BPC],
        }
        for c in range(NCORES)
    ]
    res = run_bass_kernel_spmd(nc, in_maps, core_ids=list(range(NCORES)))
    return np.concatenate([res.results[c]["out"] for c in range(NCORES)], axis=0)



# revision 38
# speedup vs baseline: 2.3646x; 1.0035x over previous
"""GarNet layer kernel for Trainium2 (8 NeuronCores, data-parallel over batch).

Math (per example b):
    w    = exp(-d_av^2)                      [V=128, S=16]
    hi   = w^T @ fi_v / V                    [S, N=64]
    out  = mean_V(w)[:, None] * hi           [S, N] -> flattened [S*N]

Implementation notes (engine-balanced, pair-packed):
  - Batch B=4096 is sharded 512/core across 8 cores (pure data parallel).
  - fi is loaded in a v-pair layout: partition P<64 holds even example
    e0=2u, P>=64 holds odd example e1=2u+1, with v = 2*(P%64)+t for slot
    t in {0,1}. Each (e, v-pair) is a 512-byte contiguous DRAM run, so
    the fi DMA runs at full modeled bandwidth (no <512B penalty).
  - One fp32 matmul per (pair, slot): the stationary operand is a
    block-diagonal [128, 32] tile (w(e0) in cols 0:16 on P<64, w(e1) in
    cols 16:32 on P>=64, zeros elsewhere), so a single 64-row moving
    stream computes BOTH examples of the pair. A second 1-row matmul
    against a constant 1/V^2 column accumulates sum_V(w)/V^2 into psum
    col 64, and the epilogue is psum[:, :64] * psum[:, 64] per partition.
  - DMAs are spread across engines (fi on SP, d_av + out on GpSimd/Pool,
    a few startup transfers on Act) so no single engine serializes the
    HBM traffic; the PE (55.5us of fp32 matmul, the structural floor) is
    the only bottleneck engine.
  - Pipeline shaping: small first/last chunks (8/24/24/...32.../8), a
    PE p-state warmup (dummy matmuls so the clock is at 2.4GHz before the
    first real matmul), an early Exp-table preload on Act, and a 3-deep
    stationary-w ring keep the PE gap-free from ~3.4us to the drain.
"""

import numpy as np
from contextlib import ExitStack

import concourse.bass as bass
import concourse.tile as tile
from concourse import mybir
from concourse.bass_utils import run_bass_kernel_spmd

B, V, S, N = 4096, 128, 16, 64
NCORES = 8
BPC = B // NCORES            # examples per core
ONES_VAL = 1.0 / (V * V)     # exact power of two; folds /V^2 into the matmul


def split_multi_waits(nc):
    """The walrus build in this container rejects >1 embedded sem-wait per
    instruction ("Too many sync wait commands" in setupSyncWait). Hoist every
    multi-wait list onto single-wait EventSemaphore instructions immediately
    before the owner on the same engine — identical semantics, since engine
    streams are in order."""
    fn = nc.m.functions[0]
    for block in fn.blocks:
        insts = list(block.instructions)
        changed = False
        new = []
        for inst in insts:
            si = inst.sync_info
            waits = list(si.on_wait) if (si and si.on_wait) else []
            if len(waits) > 1:
                changed = True
                for w in waits:
                    ev = mybir.InstEventSemaphore(
                        name=nc.get_next_instruction_name(), ins=[], outs=[]
                    )
                    ev.engine = inst.engine
                    ev.sync_info = mybir.SyncInfo(on_wait=[w], on_update=[])
                    new.append(ev)
                ups = list(si.on_update) if si.on_update else []
                inst.sync_info = mybir.SyncInfo(on_wait=[], on_update=ups)
            new.append(inst)
        if changed:
            block.instructions = new


def build(bpc=BPC, e_chunk=32, name="garnet", split_waits=True, fi_bufs=3, wb=3,
          warm=True, edge=8, warm_rows=48, warm_n=18,
          fi_act_set=(0,), fi_bufs_override=None, plan=None):
    """Build the per-core Bass module for a shard of `bpc` examples."""
    # Chunk plan: small first/last chunks shrink pipeline fill and drain.
    if plan is None:
        if edge and bpc > 3 * e_chunk:
            # Ramp in with three partial chunks (short fill), ramp out with
            # one small chunk (short drain); tuned on the CoreSim timeline.
            first = [edge, e_chunk - edge, e_chunk - edge]
            mid = (bpc - sum(first) - edge) // e_chunk
            plan = first + [e_chunk] * mid + [edge]
            rem = bpc - sum(plan)
            if rem:
                plan.insert(3, rem)
        else:
            plan = [e_chunk] * (bpc // e_chunk)
    assert sum(plan) == bpc and all(p % 8 == 0 for p in plan)
    assert all(p <= e_chunk for p in plan)
    U = e_chunk // 2          # max pairs per chunk (pool slot size)

    nc = bass.Bass(name=name)
    fi = nc.dram_tensor("fi_v", (bpc, V, N), mybir.dt.float32, kind="ExternalInput")
    dav = nc.dram_tensor("d_av", (bpc, V, S), mybir.dt.float32, kind="ExternalInput")
    out = nc.dram_tensor("out", (bpc, S * N), mybir.dt.float32, kind="ExternalOutput")

    f32 = mybir.dt.float32
    with tile.TileContext(nc) as tc, ExitStack() as ctx:
        fipool = ctx.enter_context(
            tc.tile_pool(name="fipool", bufs=fi_bufs_override or fi_bufs)
        )
        dpool = ctx.enter_context(tc.tile_pool(name="dpool", bufs=2))
        opool = ctx.enter_context(tc.tile_pool(name="opool", bufs=3))
        colpool = ctx.enter_context(tc.tile_pool(name="colpool", bufs=4))
        constpool = ctx.enter_context(tc.tile_pool(name="constpool", bufs=1))
        wpool = ctx.enter_context(tc.tile_pool(name="wpool", bufs=1))
        psum = ctx.enter_context(
            tc.tile_pool(name="psum", bufs=(7 if warm_rows and warm_n else 8), space="PSUM")
        )
        psumw = (
            ctx.enter_context(tc.tile_pool(name="psumw", bufs=1, space="PSUM"))
            if warm_rows and warm_n
            else None
        )

        WB = wb
        ones_t = constpool.tile([128, 1], f32)
        nc.vector.memset(ones_t, ONES_VAL)

        if warm:
            # Load the Act engine's Exp table during chunk-0 DMAs instead of
            # on the first real exp.
            warm_t = constpool.tile([128, 1], f32)
            nc.scalar.activation(
                warm_t, ones_t, mybir.ActivationFunctionType.Exp, scale=-1.0
            )

        # Persistent stationary-w ring. Off-diagonal zero blocks are memset
        # once; buffer 0 first so chunk 0 is not blocked behind the full-ring
        # memset (the tile scheduler keeps per-engine program order).
        w_ring = wpool.tile([128, WB, U, 2, 32], f32)
        nc.vector.memset(w_ring[0:64, 0, :, :, 16:32], 0.0)
        nc.vector.memset(w_ring[64:128, 0, :, :, 0:16], 0.0)

        starts = [sum(plan[:i]) for i in range(len(plan))]
        dma_eng = [nc.sync, nc.gpsimd]

        def load_fi(c):
            uc = plan[c] // 2
            b0 = starts[c]
            fi_t = fipool.tile([128, U, 128], f32)
            (nc.scalar if c in fi_act_set else nc.sync).dma_start(
                out=fi_t[:, 0:uc, :],
                in_=fi[b0 : b0 + plan[c]].rearrange(
                    "(u e2) (p vp) n -> (e2 p) u (vp n)", e2=2, vp=2
                ),
            )
            return fi_t

        def load_w(c):
            uc = plan[c] // 2
            b0 = starts[c]
            wbuf = c % WB
            d_t = dpool.tile([128, U, 2, 16], f32)
            dma_eng[0 if c == 0 else 1].dma_start(
                out=d_t[:, 0:uc, :, :],
                in_=dav[b0 : b0 + plan[c]].rearrange(
                    "(u e2) (p vp) s -> (e2 p) u (vp s)", e2=2, vp=2
                ),
            )
            nc.vector.tensor_mul(d_t[:, 0:uc], d_t[:, 0:uc], d_t[:, 0:uc])
            nc.scalar.activation(
                w_ring[0:64, wbuf, 0:uc, :, 0:16],
                d_t[0:64, 0:uc],
                mybir.ActivationFunctionType.Exp,
                scale=-1.0,
            )
            nc.scalar.activation(
                w_ring[64:128, wbuf, 0:uc, :, 16:32],
                d_t[64:128, 0:uc],
                mybir.ActivationFunctionType.Exp,
                scale=-1.0,
            )
            return d_t

        # PE p-state warmup: dummy matmuls keep the PE busy through the
        # chunk-0 fill so the ramp (full clock after 3us of activity) is
        # complete before the first real matmul. warm_rows*warm_n ~ fill ns.
        if warm_rows and warm_n:
            dummy_t = constpool.tile([128, warm_rows], f32)
            nc.gpsimd.memset(dummy_t, 0.0)
            wps = psumw.tile([128, N + 1], f32)
            for _ in range(warm_n):
                nc.tensor.matmul(
                    out=wps[0:1, 0:warm_rows],
                    lhsT=ones_t,
                    rhs=dummy_t,
                    start=True,
                    stop=True,
                    skip_group_check=True,
                )

        fi_tiles = {0: load_fi(0)}
        load_w(0)

        chunk_otile = {}
        for c in range(len(plan)):
            wbuf = c % WB
            G = plan[c] // 8
            if c + 1 < len(plan):
                load_w(c + 1)
                fi_tiles[c + 1] = load_fi(c + 1)
            if c + 1 < WB:
                # Ring buffer c+1's zero blocks: one buffer per iteration,
                # emitted after that chunk's loads so they never delay the
                # d->sq->exp critical chain (Pool has slack here).
                nc.gpsimd.memset(w_ring[0:64, c + 1, :, :, 16:32], 0.0)
                nc.gpsimd.memset(w_ring[64:128, c + 1, :, :, 0:16], 0.0)
            fi_t = fi_tiles.pop(c)
            o_t = opool.tile([128, G, N], f32)

            for g in range(G):
                ps = psum.tile([128, N + 1], f32)
                for j in range(4):
                    u = g * 4 + j
                    # One accumulation group per 32-row block covering BOTH
                    # the [0:N] cols and the wbar col N: start only on the
                    # first matmul, stop only on the last, so each 2KB psum
                    # zero region is started exactly once.
                    for t in range(2):
                        nc.tensor.matmul(
                            out=ps[32 * j : 32 * j + 32, 0:N],
                            lhsT=w_ring[:, wbuf, u, t, :],
                            rhs=fi_t[:, u, 64 * t : 64 * t + 64],
                            start=(t == 0),
                            stop=False,
                            tile_position=(0, 32 * j),
                            skip_group_check=True,
                        )
                        nc.tensor.matmul(
                            out=ps[32 * j : 32 * j + 32, N : N + 1],
                            lhsT=w_ring[:, wbuf, u, t, :],
                            rhs=ones_t,
                            start=False,
                            stop=(t == 1),
                            tile_position=(0, 32 * j),
                            skip_group_check=True,
                        )
                # Both epilogue ops on DVE: the col extract (immediate-scalar
                # multiply by 1.0) and the wbar multiply stay on one engine,
                # removing a cross-engine hop from the drain critical path.
                col = colpool.tile([128, 1], f32)
                nc.vector.tensor_scalar_mul(col, ps[:, N : N + 1], 1.0)
                nc.vector.tensor_scalar_mul(o_t[:, g, :], ps[:, 0:N], col)

            # psum partition 32j+16t+s belongs to example b0+8g+2j+t, row s.
            # The last chunk's store goes via SP (idle by then, and with a
            # slightly shorter DGE init) to shorten the drain.
            (nc.sync if c == len(plan) - 1 else nc.gpsimd).dma_start(
                out=out[starts[c] : starts[c] + plan[c]].rearrange(
                    "(g j t) (s n) -> (j t s) g n", j=4, t=2, s=S
                ),
                in_=o_t,
            )

    if split_waits:
        split_multi_waits(nc)
    return nc


_NC_CACHE = {}


def _get_nc():
    if "nc" not in _NC_CACHE:
        _NC_CACHE["nc"] = build()
    return _NC_CACHE["nc"]


def kernel(fi_v: np.ndarray, d_av: np.ndarray) -> np.ndarray:
    fi_v = np.ascontiguousarray(np.asarray(fi_v, dtype=np.float32))
    d_av = np.ascontiguousarray(np.asarray(d_av, dtype=np.float32))
    assert fi_v.shape == (B, V, N) and d_av.shape == (B, V, S)
    nc = _get_nc()
    in_maps = [
        {
            "fi_v": fi_v[c * BPC : (c + 1) * BPC],
            "d_av": d_av[c * BPC : (c + 1) * BPC],
        }
        for c in range(NCORES)
    ]
    res = run_bass_kernel_spmd(nc, in_maps, core_ids=list(range(NCORES)))
    return np.concatenate([res.results[c]["out"] for c in range(NCORES)], axis=0)



# revision 40
# speedup vs baseline: 2.3826x; 1.0076x over previous
"""GarNet layer kernel for Trainium2 (8 NeuronCores, data-parallel over batch).

Math (per example b):
    w    = exp(-d_av^2)                      [V=128, S=16]
    hi   = w^T @ fi_v / V                    [S, N=64]
    out  = mean_V(w)[:, None] * hi           [S, N] -> flattened [S*N]

Implementation notes (engine-balanced, pair-packed):
  - Batch B=4096 is sharded 512/core across 8 cores (pure data parallel).
  - fi is loaded in a v-pair layout: partition P<64 holds even example
    e0=2u, P>=64 holds odd example e1=2u+1, with v = 2*(P%64)+t for slot
    t in {0,1}. Each (e, v-pair) is a 512-byte contiguous DRAM run, so
    the fi DMA runs at full modeled bandwidth (no <512B penalty).
  - One fp32 matmul per (pair, slot): the stationary operand is a
    block-diagonal [128, 32] tile (w(e0) in cols 0:16 on P<64, w(e1) in
    cols 16:32 on P>=64, zeros elsewhere), so a single 64-row moving
    stream computes BOTH examples of the pair. A second 1-row matmul
    against a constant 1/V^2 column accumulates sum_V(w)/V^2 into psum
    col 64, and the epilogue is psum[:, :64] * psum[:, 64] per partition.
  - DMAs are spread across engines (fi on SP, d_av + out on GpSimd/Pool,
    a few startup transfers on Act) so no single engine serializes the
    HBM traffic; the PE (55.5us of fp32 matmul, the structural floor) is
    the only bottleneck engine.
  - Pipeline shaping: small first/last chunks (8/24/24/...32.../8), a
    PE p-state warmup (dummy matmuls so the clock is at 2.4GHz before the
    first real matmul), an early Exp-table preload on Act, and a 3-deep
    stationary-w ring keep the PE gap-free from ~3.4us to the drain.
"""

import numpy as np
from contextlib import ExitStack

import concourse.bass as bass
import concourse.tile as tile
from concourse import mybir
from concourse.bass_utils import run_bass_kernel_spmd

B, V, S, N = 4096, 128, 16, 64
NCORES = 8
BPC = B // NCORES            # examples per core
ONES_VAL = 1.0 / (V * V)     # exact power of two; folds /V^2 into the matmul


def split_multi_waits(nc):
    """The walrus build in this container rejects >1 embedded sem-wait per
    instruction ("Too many sync wait commands" in setupSyncWait). Hoist every
    multi-wait list onto single-wait EventSemaphore instructions immediately
    before the owner on the same engine — identical semantics, since engine
    streams are in order."""
    fn = nc.m.functions[0]
    for block in fn.blocks:
        insts = list(block.instructions)
        changed = False
        new = []
        for inst in insts:
            si = inst.sync_info
            waits = list(si.on_wait) if (si and si.on_wait) else []
            if len(waits) > 1:
                changed = True
                for w in waits:
                    ev = mybir.InstEventSemaphore(
                        name=nc.get_next_instruction_name(), ins=[], outs=[]
                    )
                    ev.engine = inst.engine
                    ev.sync_info = mybir.SyncInfo(on_wait=[w], on_update=[])
                    new.append(ev)
                ups = list(si.on_update) if si.on_update else []
                inst.sync_info = mybir.SyncInfo(on_wait=[], on_update=ups)
            new.append(inst)
        if changed:
            block.instructions = new


def build(bpc=BPC, e_chunk=32, name="garnet", split_waits=True, fi_bufs=3, wb=3,
          warm=True, edge=8, warm_rows=48, warm_n=18,
          fi_act_set=(0,), fi_bufs_override=None, plan=None):
    """Build the per-core Bass module for a shard of `bpc` examples."""
    # Chunk plan: small first/last chunks shrink pipeline fill and drain.
    if plan is None:
        if edge and bpc > 3 * e_chunk:
            # Ramp in with three partial chunks (short fill), ramp out with
            # one small chunk (short drain); tuned on the CoreSim timeline.
            first = [edge, e_chunk - edge, e_chunk - edge]
            mid = (bpc - sum(first) - edge) // e_chunk
            plan = first + [e_chunk] * mid + [edge]
            rem = bpc - sum(plan)
            if rem:
                plan.insert(3, rem)
        else:
            plan = [e_chunk] * (bpc // e_chunk)
    assert sum(plan) == bpc and all(p % 8 == 0 for p in plan)
    assert all(p <= e_chunk for p in plan)
    U = e_chunk // 2          # max pairs per chunk (pool slot size)

    nc = bass.Bass(name=name)
    fi = nc.dram_tensor("fi_v", (bpc, V, N), mybir.dt.float32, kind="ExternalInput")
    dav = nc.dram_tensor("d_av", (bpc, V, S), mybir.dt.float32, kind="ExternalInput")
    out = nc.dram_tensor("out", (bpc, S * N), mybir.dt.float32, kind="ExternalOutput")

    f32 = mybir.dt.float32
    with tile.TileContext(nc) as tc, ExitStack() as ctx:
        fipool = ctx.enter_context(
            tc.tile_pool(name="fipool", bufs=fi_bufs_override or fi_bufs)
        )
        dpool = ctx.enter_context(tc.tile_pool(name="dpool", bufs=2))
        opool = ctx.enter_context(tc.tile_pool(name="opool", bufs=3))
        colpool = ctx.enter_context(tc.tile_pool(name="colpool", bufs=4))
        constpool = ctx.enter_context(tc.tile_pool(name="constpool", bufs=1))
        wpool = ctx.enter_context(tc.tile_pool(name="wpool", bufs=1))
        psum = ctx.enter_context(
            tc.tile_pool(name="psum", bufs=(7 if warm_rows and warm_n else 8), space="PSUM")
        )
        psumw = (
            ctx.enter_context(tc.tile_pool(name="psumw", bufs=1, space="PSUM"))
            if warm_rows and warm_n
            else None
        )

        WB = wb
        ones_t = constpool.tile([128, 1], f32)
        nc.vector.memset(ones_t, ONES_VAL)

        if warm:
            # Load the Act engine's Exp table during chunk-0 DMAs instead of
            # on the first real exp.
            warm_t = constpool.tile([128, 1], f32)
            nc.scalar.activation(
                warm_t, ones_t, mybir.ActivationFunctionType.Exp, scale=-1.0
            )

        # Persistent stationary-w ring. Off-diagonal zero blocks are memset
        # once; buffer 0 first so chunk 0 is not blocked behind the full-ring
        # memset (the tile scheduler keeps per-engine program order).
        w_ring = wpool.tile([128, WB, U, 2, 32], f32)
        nc.vector.memset(w_ring[0:64, 0, :, :, 16:32], 0.0)
        nc.vector.memset(w_ring[64:128, 0, :, :, 0:16], 0.0)

        starts = [sum(plan[:i]) for i in range(len(plan))]
        dma_eng = [nc.sync, nc.gpsimd]

        def load_fi(c):
            uc = plan[c] // 2
            b0 = starts[c]
            fi_t = fipool.tile([128, U, 130], f32)
            nc.vector.memset(fi_t[:, 0:uc, 0:1], ONES_VAL)
            nc.vector.memset(fi_t[:, 0:uc, 129:130], ONES_VAL)
            (nc.scalar if c in fi_act_set else nc.sync).dma_start(
                out=fi_t[:, 0:uc, 1:129],
                in_=fi[b0 : b0 + plan[c]].rearrange(
                    "(u e2) (p vp) n -> (e2 p) u (vp n)", e2=2, vp=2
                ),
            )
            return fi_t

        def load_w(c):
            uc = plan[c] // 2
            b0 = starts[c]
            wbuf = c % WB
            d_t = dpool.tile([128, U, 2, 16], f32)
            dma_eng[0 if c == 0 else 1].dma_start(
                out=d_t[:, 0:uc, :, :],
                in_=dav[b0 : b0 + plan[c]].rearrange(
                    "(u e2) (p vp) s -> (e2 p) u (vp s)", e2=2, vp=2
                ),
            )
            nc.vector.tensor_mul(d_t[:, 0:uc], d_t[:, 0:uc], d_t[:, 0:uc])
            nc.scalar.activation(
                w_ring[0:64, wbuf, 0:uc, :, 0:16],
                d_t[0:64, 0:uc],
                mybir.ActivationFunctionType.Exp,
                scale=-1.0,
            )
            nc.scalar.activation(
                w_ring[64:128, wbuf, 0:uc, :, 16:32],
                d_t[64:128, 0:uc],
                mybir.ActivationFunctionType.Exp,
                scale=-1.0,
            )
            return d_t

        # PE p-state warmup: dummy matmuls keep the PE busy through the
        # chunk-0 fill so the ramp (full clock after 3us of activity) is
        # complete before the first real matmul. warm_rows*warm_n ~ fill ns.
        if warm_rows and warm_n:
            dummy_t = constpool.tile([128, warm_rows], f32)
            nc.gpsimd.memset(dummy_t, 0.0)
            wps = psumw.tile([128, N + 1], f32)
            for _ in range(warm_n):
                nc.tensor.matmul(
                    out=wps[0:1, 0:warm_rows],
                    lhsT=ones_t,
                    rhs=dummy_t,
                    start=True,
                    stop=True,
                    skip_group_check=True,
                )

        fi_tiles = {0: load_fi(0)}
        load_w(0)

        chunk_otile = {}
        for c in range(len(plan)):
            wbuf = c % WB
            G = plan[c] // 8
            if c + 1 < len(plan):
                load_w(c + 1)
                fi_tiles[c + 1] = load_fi(c + 1)
            if c + 1 < WB:
                # Ring buffer c+1's zero blocks: one buffer per iteration,
                # emitted after that chunk's loads so they never delay the
                # d->sq->exp critical chain (Pool has slack here).
                nc.gpsimd.memset(w_ring[0:64, c + 1, :, :, 16:32], 0.0)
                nc.gpsimd.memset(w_ring[64:128, c + 1, :, :, 0:16], 0.0)
            fi_t = fi_tiles.pop(c)
            o_t = opool.tile([128, G, N], f32)

            for g in range(G):
                ps = psum.tile([128, N + 2], f32)
                for j in range(4):
                    u = g * 4 + j
                    nc.tensor.matmul(
                        out=ps[32 * j : 32 * j + 32, 0 : N + 1],
                        lhsT=w_ring[:, wbuf, u, 0, :],
                        rhs=fi_t[:, u, 0:65],
                        start=True,
                        stop=False,
                        tile_position=(0, 32 * j),
                        skip_group_check=True,
                    )
                    nc.tensor.matmul(
                        out=ps[32 * j : 32 * j + 32, 1 : N + 1],
                        lhsT=w_ring[:, wbuf, u, 1, :],
                        rhs=fi_t[:, u, 65:129],
                        start=False,
                        stop=False,
                        tile_position=(0, 32 * j),
                        skip_group_check=True,
                    )
                    nc.tensor.matmul(
                        out=ps[32 * j : 32 * j + 32, N + 1 : N + 2],
                        lhsT=w_ring[:, wbuf, u, 1, :],
                        rhs=fi_t[:, u, 129:130],
                        start=False,
                        stop=True,
                        tile_position=(0, 32 * j),
                        skip_group_check=True,
                    )
                # wbar = psum col 0 + col N+1; only one PSUM input allowed
                # per vector instruction, so stage col 0 through SBUF.
                colA = colpool.tile([128, 1], f32)
                nc.vector.tensor_scalar_mul(colA, ps[:, 0:1], 1.0)
                col = colpool.tile([128, 1], f32)
                nc.vector.tensor_add(col, ps[:, N + 1 : N + 2], colA)
                nc.vector.tensor_scalar_mul(o_t[:, g, :], ps[:, 1 : N + 1], col)

            # psum partition 32j+16t+s belongs to example b0+8g+2j+t, row s.
            # The last chunk's store goes via SP (idle by then, and with a
            # slightly shorter DGE init) to shorten the drain.
            (nc.sync if c == len(plan) - 1 else nc.gpsimd).dma_start(
                out=out[starts[c] : starts[c] + plan[c]].rearrange(
                    "(g j t) (s n) -> (j t s) g n", j=4, t=2, s=S
                ),
                in_=o_t,
            )

    if split_waits:
        split_multi_waits(nc)
    return nc


_NC_CACHE = {}


def _get_nc():
    if "nc" not in _NC_CACHE:
        _NC_CACHE["nc"] = build()
    return _NC_CACHE["nc"]


def kernel(fi_v: np.ndarray, d_av: np.ndarray) -> np.ndarray:
    fi_v = np.ascontiguousarray(np.asarray(fi_v, dtype=np.float32))
    d_av = np.ascontiguousarray(np.asarray(d_av, dtype=np.float32))
    assert fi_v.shape == (B, V, N) and d_av.shape == (B, V, S)
    nc = _get_nc()
    in_maps = [
        {
            "fi_v": fi_v[c * BPC : (c + 1) * BPC],
            "d_av": d_av[c * BPC : (c + 1) * BPC],
        }
        for c in range(NCORES)
    ]
    res = run_bass_kernel_spmd(nc, in_maps, core_ids=list(range(NCORES)))
    return np.concatenate([res.results[c]["out"] for c in range(NCORES)], axis=0)

